# revision 66
# baseline (speedup 1.0000x reference)
"""DGCNN-style GNN (2x dynamic-kNN EdgeConv + global pool + MLP head) on 8 Trainium2
NeuronCores, data-parallel over the 512 graphs (64 graphs per core).

Self-contained: hardcodes all shapes; host side only does layout prep (transpose /
tiling / block-diag packing of weights) and sharding.

v5: gather volume paces the kernel (~0.65 words/ns/core on gpsimd), so gathers are
full-set (1 per conv per set, best rate) and a 4-stage software pipeline
(A(s) | B1(s-1) B2(s-1) | C(s-2)) keeps gpsimd fed back-to-back. Score matmuls are
2-term accumulating matmuls (K=2/K=16 x.x plus a rank-1 sq-row term), eliminating
the a4/b4/s18 staging tiles. conv2 gathers x1_j and computes q post-gather.
Non-score matmuls run in float32r (1 cycle/row).
"""

import sys

for _p in ("/opt/trn_rl_repo",):
    if _p not in sys.path:
        sys.path.append(_p)

from contextlib import ExitStack

import numpy as np

import concourse.bass as bass
import concourse.tile as tile
from concourse import bacc, mybir
from concourse.bass_utils import run_bass_kernel_spmd

F32 = mybir.dt.float32
F32R = mybir.dt.float32r
BF16 = mybir.dt.bfloat16
U16 = mybir.dt.uint16
I16 = mybir.dt.int16
AF = mybir.ActivationFunctionType
ALU = mybir.AluOpType
AX = mybir.AxisListType

B, N, K = 512, 256, 20
NCORES = 8
GPC = B // NCORES  # graphs per core = 64
NEG = -1.0e30


def build_program(sets: int = 8):
    G = 8 * sets
    nc = bacc.Bacc("TRN2", target_bir_lowering=False, debug=False)

    def din(name, shape, dtype=F32):
        return nc.declare_dram_parameter(name, list(shape), dtype, isOutput=False)

    # -------------------- DRAM parameters --------------------
    xtf_d = din("xtf", [128, 256])
    xrows_d = din("xrows", [2, 16384])      # [f, 256g+j] = x
    cw2_d = din("cw2", [2, 16])             # c1_w0[:2] - c1_w0[2:4]
    b0c_d = din("b0c", [16, 1])             # c1_b0
    bwrep_d = din("bwrep", [2, 128])        # tile8(c1_w0[2:4])
    w1bd_d = din("w1bd", [128, 128])        # blkdiag8(c1_w1)
    b1rep_d = din("b1rep", [128, 1])
    w2bd_d = din("w2bd", [128, 128])        # blkdiag8(c1_w2)
    b2rep_d = din("b2rep", [128, 1])
    bdgf_d = din("bdgf", [128, 64])
    nh16_d = din("neghalf16", [16, 1])
    ones1_d = din("ones1", [1, 128])
    goffs_d = din("goffs", [128, 320], I16)  # [:, 40g+j] = 256*g
    qbd_a_d = din("qbdA", [128, 128])       # blkdiag8(c2_w0[16:32, :16])
    qbd_b_d = din("qbdB", [128, 128])       # blkdiag8(c2_w0[16:32, 16:])
    wd16_d = din("wd16", [16, 32])          # c2_w0[:16]-c2_w0[16:32]
    b2c2_d = din("b2c2", [32, 1])           # c2_b0
    w1l_d = din("w1l", [48, 128])           # lin1_w, rows = [x2(32); x1(16)]
    b1l_d = din("b1l", [128, 1])
    mw0_d = din("mw0", [128, 64])
    mb0_d = din("mb0", [64, 1])
    mw1_d = din("mw1", [64, 64])
    mb1_d = din("mb1", [64, 1])
    mw2_d = din("mw2", [64, 1])
    mb2_d = din("mb2", [1, 1])
    out_d = nc.declare_dram_parameter("out", [1, G], F32, isOutput=True)

    with tile.TileContext(nc) as tc, ExitStack() as ctx:
        P = lambda **kw: ctx.enter_context(tc.tile_pool(**kw))
        wp = P(name="weights", bufs=1)

        def load(dram, shape, dtype=F32):
            t = wp.tile(list(shape), dtype, tag=dram.name)
            src = dram.ap()
            if dtype == F32R:
                src = src.bitcast(F32R)
            nc.sync.dma_start(t[:], src)
            return t

        xtf = load(xtf_d, [128, 256])
        cw2 = load(cw2_d, [2, 16], F32R)
        b0c = load(b0c_d, [16, 1])
        bwrep = load(bwrep_d, [2, 128], F32R)
        w1bd = load(w1bd_d, [128, 128], F32R)
        b1rep = load(b1rep_d, [128, 1])
        w2bd = load(w2bd_d, [128, 128], F32R)
        b2rep = load(b2rep_d, [128, 1])
        bdgf = load(bdgf_d, [128, 64])
        nh16 = load(nh16_d, [16, 1], F32R)
        ones1 = load(ones1_d, [1, 128])
        goffs = load(goffs_d, [128, 320], I16)
        qbd_a = load(qbd_a_d, [128, 128])
        qbd_b = load(qbd_b_d, [128, 128])
        wd16 = load(wd16_d, [16, 32])
        b2c2 = load(b2c2_d, [32, 1])
        w1l = load(w1l_d, [48, 128], F32R)
        b1l = load(b1l_d, [128, 1])
        mw0 = load(mw0_d, [128, 64])
        mb0 = load(mb0_d, [64, 1])
        mw1 = load(mw1_d, [64, 64])
        mb1 = load(mb1_d, [64, 1])
        mw2 = load(mw2_d, [64, 1])
        mb2 = load(mb2_d, [1, 1])

        pooledT = wp.tile([128, G], F32)

        # PSUM pools: 2 + 2 + 4 banks = 8
        pl_sc = P(name="scps", bufs=2, space="PSUM")      # [128,256] score psums
        pl_ml = P(name="mlps", bufs=2, space="PSUM")      # [128,320] mlp/q psums
        pl_bg = P(name="bgps", bufs=4, space="PSUM")      # [128,512] everything else

        sc_p = P(name="scores", bufs=3)
        v_p = P(name="vals8", bufs=3)
        ix_p = P(name="idx", bufs=2)
        h_p = P(name="hid", bufs=2)
        set_p = P(name="sets", bufs=2)
        f48_p = P(name="f48", bufs=2)
        big1_p = P(name="big1", bufs=1)
        bt2_p = P(name="bt2", bufs=2)
        x1r_p = P(name="x1r", bufs=1)
        sqc_p = P(name="sqc", bufs=3)
        gp_p = P(name="gath", bufs=2)       # full-set gather outputs [128,5120]

        # ---------------- global prep: sqG = -0.5*(x0^2+x1^2) [64,256] ----------
        xsq = wp.tile([128, 256], F32, tag="xsq")
        nc.vector.tensor_tensor(out=xsq[:], in0=xtf[:], in1=xtf[:], op=ALU.mult)
        sq_ps = pl_bg.tile([128, 512], F32, tag="bgps")
        nc.tensor.matmul(sq_ps[0:64, 0:256], lhsT=bdgf[:], rhs=xsq[:], start=True, stop=True)
        sqG = wp.tile([64, 256], F32)
        nc.scalar.copy(sqG[:], sq_ps[0:64, 0:256])

        def topk20(scores_sb, ixp, col0):
            """scores_sb [128,256] bf16 (destroyed); writes indices of ranks 1..20
            into ixp[:, col0:col0+20] as i16."""
            ixt = ix_p.tile([128, 24], U16, tag="ix")
            for r in range(3):
                v = v_p.tile([128, 8], BF16, tag="v8")
                nc.vector.max(v[:], scores_sb[:])
                nc.vector.max_index(ixt[:, 8 * r:8 * r + 8], v[:], scores_sb[:])
                if r < 2:
                    nc.vector.match_replace(scores_sb[:], v[:], scores_sb[:], NEG)
            nc.vector.tensor_copy(out=ixp[:, col0:col0 + 20], in_=ixt[:, 1:21])

        def knn_scores(src, nch, sq_ap, g8, t, ixp):
            """src [nch, 2048] = y (base 0); sq_ap [1, 256] = -0.5|y_j|^2 for this
            graph (base 0); writes topk indices for (g8, t) into ixp."""
            ps = pl_sc.tile([128, 256], F32, tag="scps")
            nc.tensor.matmul(
                ps[:],
                lhsT=src[0:nch, 256 * g8 + 128 * t:256 * g8 + 128 * (t + 1)].bitcast(F32),
                rhs=src[0:nch, 256 * g8:256 * (g8 + 1)].bitcast(F32),
                start=True, stop=False)
            nc.tensor.matmul(ps[:], lhsT=ones1[:], rhs=sq_ap,
                             start=False, stop=True)
            sc = sc_p.tile([128, 256], BF16, tag="sc")
            nc.scalar.copy(sc[:], ps[:])
            topk20(sc, ixp, 40 * g8 + 20 * t)

        # per-set live state carried between pipelined phases
        st = [dict() for _ in range(sets)]

        # ------------------------------------------------------------------
        # Phase A(s): conv1 tables + kNN + full-set gather issue
        # ------------------------------------------------------------------
        def phase_A(s):
            d = st[s]
            xr = set_p.tile([2, 2048], F32R, tag="xr")
            nc.sync.dma_start(xr[:], xrows_d.ap()[:, 2048 * s:2048 * (s + 1)].bitcast(F32R))
            d["xr"] = xr

            # center-term table ct2set [(ng,c), (g,t,pb)]
            ctall = big1_p.tile([16, 2048], F32, tag="ctall")
            for q in range(4):
                cps = pl_bg.tile([128, 512], F32, tag="bgps")
                nc.tensor.matmul(cps[0:16, :], lhsT=cw2[:],
                                 rhs=xr[0:2, 512 * q:512 * (q + 1)], start=True, stop=True)
                nc.scalar.add(ctall[:, 512 * q:512 * (q + 1)], cps[0:16, :], b0c[:])
            ct2set = set_p.tile([128, 256], F32, tag="ct2")
            ctv = ctall[:].rearrange("c (g t ng pb) -> c g t ng pb",
                                     g=8, t=2, ng=8, pb=16)
            for ng in range(8):
                nc.sync.dma_start(ct2set[16 * ng:16 * ng + 16, :], ctv[:, :, :, ng, :])
            d["ct2set"] = ct2set

            # neighbor-term table btset [(ng,c), (g,node)]
            btset = bt2_p.tile([128, 2048], F32, tag="btset")
            for q in range(4):
                bps = pl_bg.tile([128, 512], F32, tag="bgps")
                nc.tensor.matmul(bps[:], lhsT=bwrep[:],
                                 rhs=xr[0:2, 512 * q:512 * (q + 1)], start=True, stop=True)
                nc.scalar.copy(btset[:, 512 * q:512 * (q + 1)], bps[:])

            ixp = ix_p.tile([128, 320], I16, tag="ixp1")
            for g8 in range(8):
                sqc = sqc_p.tile([1, 256], F32, tag="sqc")
                nc.sync.dma_start(sqc[:], sqG[8 * s + g8:8 * s + g8 + 1, :])
                for t in range(2):
                    knn_scores(xr, 2, sqc[:], g8, t, ixp)
            nc.vector.tensor_tensor(out=ixp[:].bitcast(U16), in0=ixp[:].bitcast(U16),
                                    in1=goffs[:].bitcast(U16), op=ALU.add)
            bgall = gp_p.tile([128, 5120], F32, tag="bgall")
            nc.gpsimd.ap_gather(bgall[:], btset[:], ixp[:],
                                channels=128, num_elems=2048, d=1, num_idxs=5120)
            d["bgall"] = bgall

        # ------------------------------------------------------------------
        # Phase B1(s): conv1 edge MLP (consumes the gather)
        # ------------------------------------------------------------------
        def phase_B1(s):
            d = st[s]
            ct2set, bgall = d["ct2set"], d["bgall"]
            x1parts = set_p.tile([128, 256], F32, tag="x1p")
            prev = None
            for g8 in range(8):
                for t in range(2):
                    ctb = ct2set[:, 32 * g8 + 16 * t:32 * g8 + 16 * (t + 1)]
                    ctb = ctb.unsqueeze(1).broadcast_to((128, 20, 16))
                    h1 = h_p.tile([128, 320], F32R, tag="h1")
                    h1v = h1[:].rearrange("p (k pb) -> p k pb", k=20, pb=16)
                    bgv = bgall[:, 640 * g8 + 320 * t:640 * g8 + 320 * (t + 1)]
                    bgv = bgv.rearrange("p (k pb) -> p k pb", k=20, pb=16)
                    nc.vector.tensor_tensor(out=h1v, in0=bgv, in1=ctb, op=ALU.add)
                    nc.vector.tensor_scalar_max(out=h1[:], in0=h1[:], scalar1=0.0)
                    m1 = pl_ml.tile([128, 320], F32, tag="mlps")
                    nc.tensor.matmul(m1[:], lhsT=w1bd[:], rhs=h1[:], start=True, stop=True)
                    h2 = h_p.tile([128, 320], F32R, tag="h2")
                    nc.scalar.activation(h2[:], m1[:], AF.Relu, bias=b1rep[:])
                    if prev is not None:
                        pm, pg, pt = prev
                        v = pm[:].rearrange("p (k pb) -> p pb k", k=20, pb=16)
                        nc.vector.tensor_reduce(
                            out=x1parts[:, 32 * pg + 16 * pt:32 * pg + 16 * (pt + 1)],
                            in_=v, axis=AX.X, op=ALU.max)
                    m2 = pl_ml.tile([128, 320], F32, tag="mlps")
                    nc.tensor.matmul(m2[:], lhsT=w2bd[:], rhs=h2[:], start=True, stop=True)
                    prev = (m2, g8, t)
            pm, pg, pt = prev
            v = pm[:].rearrange("p (k pb) -> p pb k", k=20, pb=16)
            nc.vector.tensor_reduce(
                out=x1parts[:, 32 * pg + 16 * pt:32 * pg + 16 * (pt + 1)],
                in_=v, axis=AX.X, op=ALU.max)
            nc.vector.tensor_scalar_add(out=x1parts[:], in0=x1parts[:], scalar1=b2rep[:])
            d["x1parts"] = x1parts

        # ------------------------------------------------------------------
        # Phase B2(s): x1 relayout + sq row, x1rep, conv2 kNN + gather issue
        # ------------------------------------------------------------------
        def phase_B2(s):
            d = st[s]
            x1parts = d["x1parts"]
            # x1c [16, 2048] = x1 [c, (g,t,ng,pb)]; sq2row [1, 2048] = -0.5|x1|^2
            x1c = set_p.tile([16, 2048], F32R, tag="x1c")
            x1v = x1c[:].rearrange("c (g t ng pb) -> c g t ng pb",
                                   g=8, t=2, ng=8, pb=16)
            for ng in range(8):
                nc.sync.dma_start(x1v[:, :, :, ng:ng + 1, :],
                                  x1parts[16 * ng:16 * ng + 16, :].bitcast(F32R))
            sq2row = big1_p.tile([1, 2048], F32, tag="sq2row")
            for q in range(4):
                x1sq = big1_p.tile([16, 512], F32R, tag="x1sq")
                nc.scalar.activation(x1sq[:], x1c[:, 512 * q:512 * (q + 1)], AF.Square)
                mq = pl_bg.tile([128, 512], F32, tag="bgps")
                nc.tensor.matmul(mq[0:1, :], lhsT=nh16[:], rhs=x1sq[:],
                                 start=True, stop=True)
                nc.scalar.copy(sq2row[:, 512 * q:512 * (q + 1)], mq[0:1, :])
            d["x1c"] = x1c
            d["sq2row"] = sq2row

            # x1rep [(ng,c), (g,node)] = x1 replicated into all 8 row-blocks
            x1rep = x1r_p.tile([128, 2048], F32, tag="x1rep")
            for ng in range(8):
                nc.sync.dma_start(x1rep[16 * ng:16 * ng + 16, :], x1c[:].bitcast(F32))

            f48 = f48_p.tile([48, 2048], F32R, tag="f48")
            nc.sync.dma_start(f48[32:48, :], x1c[:])
            d["f48"] = f48

            ixp = ix_p.tile([128, 320], I16, tag="ixp2")
            for g8 in range(8):
                for t in range(2):
                    knn_scores(x1c, 16, sq2row[:, 256 * g8:256 * (g8 + 1)], g8, t, ixp)
            nc.vector.tensor_tensor(out=ixp[:].bitcast(U16), in0=ixp[:].bitcast(U16),
                                    in1=goffs[:].bitcast(U16), op=ALU.add)
            xjall = gp_p.tile([128, 5120], F32, tag="xjall")
            nc.gpsimd.ap_gather(xjall[:], x1rep[:], ixp[:],
                                channels=128, num_elems=2048, d=1, num_idxs=5120)
            d["xjall"] = xjall

        # ------------------------------------------------------------------
        # Phase C(s): conv2 q-matmuls + aggregation + f48 + lin1 + pool
        # ------------------------------------------------------------------
        def phase_C(s):
            d = st[s]
            x1c, f48, xjall = d["x1c"], d["f48"], d["xjall"]
            x2pa = set_p.tile([128, 256], F32, tag="x2pa")
            x2pb = set_p.tile([128, 256], F32, tag="x2pb")
            for g8 in range(8):
                for t in range(2):
                    xv = xjall[:, 640 * g8 + 320 * t:640 * g8 + 320 * (t + 1)]
                    qa = pl_ml.tile([128, 320], F32, tag="mlps")
                    nc.tensor.matmul(qa[:], lhsT=qbd_a[:], rhs=xv, start=True, stop=True)
                    qv = qa[:].rearrange("p (k pb) -> p pb k", k=20, pb=16)
                    nc.vector.tensor_reduce(
                        out=x2pa[:, 32 * g8 + 16 * t:32 * g8 + 16 * (t + 1)],
                        in_=qv, axis=AX.X, op=ALU.max)
                    qb = pl_ml.tile([128, 320], F32, tag="mlps")
                    nc.tensor.matmul(qb[:], lhsT=qbd_b[:], rhs=xv, start=True, stop=True)
                    qv = qb[:].rearrange("p (k pb) -> p pb k", k=20, pb=16)
                    nc.vector.tensor_reduce(
                        out=x2pb[:, 32 * g8 + 16 * t:32 * g8 + 16 * (t + 1)],
                        in_=qv, axis=AX.X, op=ALU.max)
            # linear term of conv2 (+bias) -> F48 rows 0..31 directly
            for g8 in range(8):
                l2 = pl_bg.tile([128, 512], F32, tag="bgps")
                nc.tensor.matmul(l2[0:32, 0:256], lhsT=wd16[:],
                                 rhs=x1c[:, 256 * g8:256 * (g8 + 1)].bitcast(F32),
                                 start=True, stop=True)
                nc.scalar.add(f48[0:32, 256 * g8:256 * (g8 + 1)], l2[0:32, 0:256], b2c2[:])
            x2t = big1_p.tile([32, 2048], F32, tag="x2t")
            x2tv = x2t[:].rearrange("c (g t ng pb) -> c g t ng pb",
                                    g=8, t=2, ng=8, pb=16)
            for ng in range(8):
                nc.sync.dma_start(x2tv[0:16, :, :, ng:ng + 1, :],
                                  x2pa[16 * ng:16 * ng + 16, :])
                nc.sync.dma_start(x2tv[16:32, :, :, ng:ng + 1, :],
                                  x2pb[16 * ng:16 * ng + 16, :])
            nc.vector.tensor_tensor(out=f48[0:32, :], in0=f48[0:32, :],
                                    in1=x2t[:], op=ALU.add)
            for g8 in range(8):
                pl = pl_sc.tile([128, 256], F32, tag="scps")
                nc.tensor.matmul(pl[:], lhsT=w1l[:], rhs=f48[:, 256 * g8:256 * (g8 + 1)],
                                 start=True, stop=True)
                nc.vector.tensor_reduce(out=pooledT[:, 8 * s + g8:8 * s + g8 + 1],
                                        in_=pl[:], axis=AX.X, op=ALU.max)
            st[s] = {}

        # ---------------- 4-stage pipelined emission ----------------
        for i in range(sets + 2):
            if i < sets:
                phase_A(i)
            if 1 <= i <= sets:
                phase_B1(i - 1)
                phase_B2(i - 1)
            if i >= 2:
                phase_C(i - 2)

        # ---------------- head MLP ----------------
        nc.vector.tensor_scalar_add(out=pooledT[:], in0=pooledT[:], scalar1=b1l[:])
        hd1 = pl_bg.tile([128, 512], F32, tag="bgps")
        nc.tensor.matmul(hd1[0:64, 0:G], lhsT=mw0[:], rhs=pooledT[:], start=True, stop=True)
        h1s = wp.tile([64, G], F32)
        nc.scalar.activation(h1s[:], hd1[0:64, 0:G], AF.Relu, bias=mb0[:])
        hd2 = pl_bg.tile([128, 512], F32, tag="bgps")
        nc.tensor.matmul(hd2[0:64, 0:G], lhsT=mw1[:], rhs=h1s[:], start=True, stop=True)
        h2s = wp.tile([64, G], F32)
        nc.scalar.activation(h2s[:], hd2[0:64, 0:G], AF.Relu, bias=mb1[:])
        hd3 = pl_bg.tile([128, 512], F32, tag="bgps")
        nc.tensor.matmul(hd3[0:1, 0:G], lhsT=mw2[:], rhs=h2s[:], start=True, stop=True)
        outs = wp.tile([1, G], F32)
        nc.vector.tensor_scalar_add(out=outs[:], in0=hd3[0:1, 0:G], scalar1=mb2[:])
        nc.sync.dma_start(out_d.ap(), outs[:])

    nc.compile()
    return nc


# ---------------------------------------------------------------------------
# Host-side input prep
# ---------------------------------------------------------------------------

def _blkdiag8(w):
    w = np.asarray(w, np.float32)
    n, m = w.shape
    out = np.zeros((8 * n, 8 * m), np.float32)
    for i in range(8):
        out[n * i:n * i + n, m * i:m * i + m] = w
    return out


def make_in_maps(inputs):
    x = np.asarray(inputs["x"], np.float32)
    c1_w0 = np.asarray(inputs["c1_w0"], np.float32)
    consts = {}
    consts["cw2"] = (c1_w0[:2] - c1_w0[2:4]).astype(np.float32)
    consts["b0c"] = np.asarray(inputs["c1_b0"], np.float32)[:, None]
    consts["bwrep"] = np.tile(c1_w0[2:4], (1, 8)).astype(np.float32)
    consts["w1bd"] = _blkdiag8(inputs["c1_w1"])
    consts["b1rep"] = np.tile(np.asarray(inputs["c1_b1"], np.float32), 8)[:, None]
    consts["w2bd"] = _blkdiag8(inputs["c1_w2"])
    consts["b2rep"] = np.tile(np.asarray(inputs["c1_b2"], np.float32), 8)[:, None]
    bdgf = np.zeros((128, 64), np.float32)
    for g in range(64):
        bdgf[2 * g, g] = -0.5
        bdgf[2 * g + 1, g] = -0.5
    consts["bdgf"] = bdgf
    consts["neghalf16"] = np.full((16, 1), -0.5, np.float32)
    consts["ones1"] = np.ones((1, 128), np.float32)
    consts["goffs"] = np.tile(np.repeat(np.arange(8, dtype=np.int16) * 256, 40),
                              (128, 1))
    c2_w0 = np.asarray(inputs["c2_w0"], np.float32)
    consts["qbdA"] = _blkdiag8(c2_w0[16:32, 0:16])
    consts["qbdB"] = _blkdiag8(c2_w0[16:32, 16:32])
    consts["wd16"] = (c2_w0[0:16] - c2_w0[16:32]).astype(np.float32)
    consts["b2c2"] = np.asarray(inputs["c2_b0"], np.float32)[:, None]
    w1l = np.asarray(inputs["lin1_w"], np.float32)
    consts["w1l"] = np.concatenate([w1l[16:48], w1l[0:16]], axis=0)
    consts["b1l"] = np.asarray(inputs["lin1_b"], np.float32)[:, None]
    consts["mw0"] = np.asarray(inputs["m_w0"], np.float32)
    consts["mb0"] = np.asarray(inputs["m_b0"], np.float32)[:, None]
    consts["mw1"] = np.asarray(inputs["m_w1"], np.float32)
    consts["mb1"] = np.asarray(inputs["m_b1"], np.float32)[:, None]
    consts["mw2"] = np.asarray(inputs["m_w2"], np.float32)
    consts["mb2"] = np.asarray(inputs["m_b2"], np.float32)[:, None]

    in_maps = []
    npc = N * GPC
    for c in range(NCORES):
        xc = x[c * npc:(c + 1) * npc]
        xg = xc.reshape(GPC, N, 2)
        m = dict(consts)
        m["xtf"] = xg.transpose(0, 2, 1).reshape(128, 256).copy()
        m["xrows"] = xc.T.reshape(2, -1).copy()
        in_maps.append(m)
    return in_maps


_CACHED = {}


def _get_program(sets=8):
    if sets not in _CACHED:
        _CACHED[sets] = build_program(sets)
    return _CACHED[sets]


def run(inputs, trace=False, **kw):
    nc = _get_program(8)
    in_maps = make_in_maps(inputs)
    res = run_bass_kernel_spmd(nc, in_maps, list(range(NCORES)), trace=trace, **kw)
    out = np.concatenate([res.results[c]["out"].reshape(GPC) for c in range(NCORES)])
    return out.reshape(B, 1).astype(np.float32), res


def kernel(**inputs) -> np.ndarray:
    out, _ = run(inputs, trace=False)
    return out


# revision 68
# speedup vs baseline: 1.0163x; 1.0163x over previous
"""DGCNN-style GNN (2x dynamic-kNN EdgeConv + global pool + MLP head) on 8 Trainium2
NeuronCores, data-parallel over the 512 graphs (64 graphs per core).

Self-contained: hardcodes all shapes; host side only does layout prep (transpose /
tiling / block-diag packing of weights) and sharding.

v5: gather volume paces the kernel (~0.65 words/ns/core on gpsimd), so gathers are
full-set (1 per conv per set, best rate) and a 4-stage software pipeline
(A(s) | B1(s-1) B2(s-1) | C(s-2)) keeps gpsimd fed back-to-back. Score matmuls are
2-term accumulating matmuls (K=2/K=16 x.x plus a rank-1 sq-row term), eliminating
the a4/b4/s18 staging tiles. conv2 gathers x1_j and computes q post-gather.
Non-score matmuls run in float32r (1 cycle/row).
"""

import sys

for _p in ("/opt/trn_rl_repo",):
    if _p not in sys.path:
        sys.path.append(_p)

from contextlib import ExitStack

import numpy as np

import concourse.bass as bass
import concourse.tile as tile
from concourse import bacc, mybir
from concourse.bass_utils import run_bass_kernel_spmd

F32 = mybir.dt.float32
F32R = mybir.dt.float32r
BF16 = mybir.dt.bfloat16
U16 = mybir.dt.uint16
I16 = mybir.dt.int16
AF = mybir.ActivationFunctionType
ALU = mybir.AluOpType
AX = mybir.AxisListType

B, N, K = 512, 256, 20
NCORES = 8
GPC = B // NCORES  # graphs per core = 64
NEG = -1.0e30


def build_program(sets: int = 8):
    G = 8 * sets
    nc = bacc.Bacc("TRN2", target_bir_lowering=False, debug=False)

    def din(name, shape, dtype=F32):
        return nc.declare_dram_parameter(name, list(shape), dtype, isOutput=False)

    # -------------------- DRAM parameters --------------------
    xtf_d = din("xtf", [128, 256])
    xrows_d = din("xrows", [2, 16384])      # [f, 256g+j] = x
    cw2_d = din("cw2", [2, 16])             # c1_w0[:2] - c1_w0[2:4]
    b0c_d = din("b0c", [16, 1])             # c1_b0
    bwrep_d = din("bwrep", [2, 128])        # tile8(c1_w0[2:4])
    w1bd_d = din("w1bd", [128, 128])        # blkdiag8(c1_w1)
    b1rep_d = din("b1rep", [128, 1])
    w2bd_d = din("w2bd", [128, 128])        # blkdiag8(c1_w2)
    b2rep_d = din("b2rep", [128, 1])
    bdgf_d = din("bdgf", [128, 64])
    nh16b_d = din("nh16b", [16, 1], U16)    # bf16(-0.5) bits
    ones1b_d = din("ones1b", [1, 128], U16)  # bf16(1.0) bits
    goffs_d = din("goffs", [128, 320], I16)  # [:, 40g+j] = 256*g
    qbd_a_d = din("qbdA", [128, 128])       # blkdiag8(c2_w0[16:32, :16])
    qbd_b_d = din("qbdB", [128, 128])       # blkdiag8(c2_w0[16:32, 16:])
    wd16_d = din("wd16", [16, 32])          # c2_w0[:16]-c2_w0[16:32]
    b2c2_d = din("b2c2", [32, 1])           # c2_b0
    w1l_d = din("w1l", [48, 128])           # lin1_w, rows = [x2(32); x1(16)]
    b1l_d = din("b1l", [128, 1])
    mw0_d = din("mw0", [128, 64])
    mb0_d = din("mb0", [64, 1])
    mw1_d = din("mw1", [64, 64])
    mb1_d = din("mb1", [64, 1])
    mw2_d = din("mw2", [64, 1])
    mb2_d = din("mb2", [1, 1])
    out_d = nc.declare_dram_parameter("out", [1, G], F32, isOutput=True)

    with tile.TileContext(nc) as tc, ExitStack() as ctx:
        P = lambda **kw: ctx.enter_context(tc.tile_pool(**kw))
        wp = P(name="weights", bufs=1)

        def load(dram, shape, dtype=F32):
            t = wp.tile(list(shape), dtype, tag=dram.name)
            src = dram.ap()
            if dtype == F32R:
                src = src.bitcast(F32R)
            nc.sync.dma_start(t[:], src)
            return t

        xtf = load(xtf_d, [128, 256])
        cw2 = load(cw2_d, [2, 16], F32R)
        b0c = load(b0c_d, [16, 1])
        bwrep = load(bwrep_d, [2, 128], F32R)
        w1bd = load(w1bd_d, [128, 128], F32R)
        b1rep = load(b1rep_d, [128, 1])
        w2bd = load(w2bd_d, [128, 128], F32R)
        b2rep = load(b2rep_d, [128, 1])
        bdgf = load(bdgf_d, [128, 64])
        nh16b = load(nh16b_d, [16, 1], U16)
        ones1b = load(ones1b_d, [1, 128], U16)
        goffs = load(goffs_d, [128, 320], I16)
        qbd_a = load(qbd_a_d, [128, 128])
        qbd_b = load(qbd_b_d, [128, 128])
        wd16 = load(wd16_d, [16, 32])
        b2c2 = load(b2c2_d, [32, 1])
        w1l = load(w1l_d, [48, 128], F32R)
        b1l = load(b1l_d, [128, 1])
        mw0 = load(mw0_d, [128, 64])
        mb0 = load(mb0_d, [64, 1])
        mw1 = load(mw1_d, [64, 64])
        mb1 = load(mb1_d, [64, 1])
        mw2 = load(mw2_d, [64, 1])
        mb2 = load(mb2_d, [1, 1])

        pooledT = wp.tile([128, G], F32)

        # PSUM pools: 2 + 2 + 4 banks = 8
        pl_sc = P(name="scps", bufs=2, space="PSUM")      # [128,256] score psums
        pl_ml = P(name="mlps", bufs=2, space="PSUM")      # [128,320] mlp/q psums
        pl_bg = P(name="bgps", bufs=4, space="PSUM")      # [128,512] everything else

        sc_p = P(name="scores", bufs=3)
        v_p = P(name="vals8", bufs=3)
        ix_p = P(name="idx", bufs=2)
        h_p = P(name="hid", bufs=2)
        set_p = P(name="sets", bufs=2)
        f48_p = P(name="f48", bufs=2)
        big1_p = P(name="big1", bufs=1)
        bt2_p = P(name="bt2", bufs=2)
        x1r_p = P(name="x1r", bufs=1)
        sqc_p = P(name="sqc", bufs=2)
        gp_p = P(name="gath", bufs=2)       # full-set gather outputs [128,5120]

        # ---------------- global prep: sqG = -0.5*(x0^2+x1^2) [64,256] ----------
        xsq = wp.tile([128, 256], F32, tag="xsq")
        nc.vector.tensor_tensor(out=xsq[:], in0=xtf[:], in1=xtf[:], op=ALU.mult)
        sq_ps = pl_bg.tile([128, 512], F32, tag="bgps")
        nc.tensor.matmul(sq_ps[0:64, 0:256], lhsT=bdgf[:], rhs=xsq[:], start=True, stop=True)
        sqGb = wp.tile([64, 256], BF16)
        nc.scalar.copy(sqGb[:], sq_ps[0:64, 0:256])

        def topk20(scores_sb, ixp, col0):
            """scores_sb [128,256] bf16 (destroyed); writes indices of ranks 1..20
            into ixp[:, col0:col0+20] as i16."""
            ixt = ix_p.tile([128, 24], U16, tag="ix")
            for r in range(3):
                v = v_p.tile([128, 8], BF16, tag="v8")
                nc.vector.max(v[:], scores_sb[:])
                nc.vector.max_index(ixt[:, 8 * r:8 * r + 8], v[:], scores_sb[:])
                if r < 2:
                    nc.vector.match_replace(scores_sb[:], v[:], scores_sb[:], NEG)
            nc.vector.tensor_copy(out=ixp[:, col0:col0 + 20], in_=ixt[:, 1:21])

        def knn_scores(src, nch, sq_ap, g8, t, ixp):
            """src [nch, 2048] bf16 = y (base 0); sq_ap [1, 256] bf16 =
            -0.5|y_j|^2 for this graph (base 0); topk indices -> ixp."""
            ps = pl_sc.tile([128, 256], F32, tag="scps")
            nc.tensor.matmul(
                ps[:],
                lhsT=src[0:nch, 256 * g8 + 128 * t:256 * g8 + 128 * (t + 1)],
                rhs=src[0:nch, 256 * g8:256 * (g8 + 1)],
                start=True, stop=False)
            nc.tensor.matmul(ps[:], lhsT=ones1b[:].bitcast(BF16), rhs=sq_ap,
                             start=False, stop=True)
            sc = sc_p.tile([128, 256], BF16, tag="sc")
            nc.scalar.copy(sc[:], ps[:])
            topk20(sc, ixp, 40 * g8 + 20 * t)

        # per-set live state carried between pipelined phases
        st = [dict() for _ in range(sets)]

        # ------------------------------------------------------------------
        # Phase A(s): conv1 tables + kNN + full-set gather issue
        # ------------------------------------------------------------------
        def phase_A(s):
            d = st[s]
            xr = set_p.tile([2, 2048], F32R, tag="xr")
            nc.sync.dma_start(xr[:], xrows_d.ap()[:, 2048 * s:2048 * (s + 1)].bitcast(F32R))
            d["xr"] = xr

            # center-term table ct2set [(ng,c), (g,t,pb)]
            ctall = big1_p.tile([16, 2048], F32, tag="ctall")
            for q in range(4):
                cps = pl_bg.tile([128, 512], F32, tag="bgps")
                nc.tensor.matmul(cps[0:16, :], lhsT=cw2[:],
                                 rhs=xr[0:2, 512 * q:512 * (q + 1)], start=True, stop=True)
                nc.scalar.add(ctall[:, 512 * q:512 * (q + 1)], cps[0:16, :], b0c[:])
            ct2set = set_p.tile([128, 256], F32, tag="ct2")
            ctv = ctall[:].rearrange("c (g t ng pb) -> c g t ng pb",
                                     g=8, t=2, ng=8, pb=16)
            for ng in range(8):
                nc.sync.dma_start(ct2set[16 * ng:16 * ng + 16, :], ctv[:, :, :, ng, :])
            d["ct2set"] = ct2set

            # neighbor-term table btset [(ng,c), (g,node)]
            btset = bt2_p.tile([128, 2048], F32, tag="btset")
            for q in range(4):
                bps = pl_bg.tile([128, 512], F32, tag="bgps")
                nc.tensor.matmul(bps[:], lhsT=bwrep[:],
                                 rhs=xr[0:2, 512 * q:512 * (q + 1)], start=True, stop=True)
                nc.scalar.copy(btset[:, 512 * q:512 * (q + 1)], bps[:])

            xrb = big1_p.tile([2, 2048], BF16, tag="xrb")
            nc.scalar.copy(xrb[:], xr[:])
            ixp = ix_p.tile([128, 320], I16, tag="ixp1")
            for g8 in range(8):
                sqc = sqc_p.tile([1, 256], BF16, tag="sqc")
                nc.sync.dma_start(sqc[:], sqGb[8 * s + g8:8 * s + g8 + 1, :])
                for t in range(2):
                    knn_scores(xrb, 2, sqc[:], g8, t, ixp)
            nc.vector.tensor_tensor(out=ixp[:].bitcast(U16), in0=ixp[:].bitcast(U16),
                                    in1=goffs[:].bitcast(U16), op=ALU.add)
            bgall = gp_p.tile([128, 5120], F32, tag="bgall")
            nc.gpsimd.ap_gather(bgall[:], btset[:], ixp[:],
                                channels=128, num_elems=2048, d=1, num_idxs=5120)
            d["bgall"] = bgall

        # ------------------------------------------------------------------
        # Phase B1(s): conv1 edge MLP (consumes the gather)
        # ------------------------------------------------------------------
        def phase_B1(s):
            d = st[s]
            ct2set, bgall = d["ct2set"], d["bgall"]
            x1parts = set_p.tile([128, 256], F32, tag="x1p")
            prev = None
            for g8 in range(8):
                for t in range(2):
                    ctb = ct2set[:, 32 * g8 + 16 * t:32 * g8 + 16 * (t + 1)]
                    ctb = ctb.unsqueeze(1).broadcast_to((128, 20, 16))
                    h1 = h_p.tile([128, 320], F32R, tag="h1")
                    h1v = h1[:].rearrange("p (k pb) -> p k pb", k=20, pb=16)
                    bgv = bgall[:, 640 * g8 + 320 * t:640 * g8 + 320 * (t + 1)]
                    bgv = bgv.rearrange("p (k pb) -> p k pb", k=20, pb=16)
                    nc.vector.tensor_tensor(out=h1v, in0=bgv, in1=ctb, op=ALU.add)
                    nc.vector.tensor_scalar_max(out=h1[:], in0=h1[:], scalar1=0.0)
                    m1 = pl_ml.tile([128, 320], F32, tag="mlps")
                    nc.tensor.matmul(m1[:], lhsT=w1bd[:], rhs=h1[:], start=True, stop=True)
                    h2 = h_p.tile([128, 320], F32R, tag="h2")
                    nc.scalar.activation(h2[:], m1[:], AF.Relu, bias=b1rep[:])
                    if prev is not None:
                        pm, pg, pt = prev
                        v = pm[:].rearrange("p (k pb) -> p pb k", k=20, pb=16)
                        nc.vector.tensor_reduce(
                            out=x1parts[:, 32 * pg + 16 * pt:32 * pg + 16 * (pt + 1)],
                            in_=v, axis=AX.X, op=ALU.max)
                    m2 = pl_ml.tile([128, 320], F32, tag="mlps")
                    nc.tensor.matmul(m2[:], lhsT=w2bd[:], rhs=h2[:], start=True, stop=True)
                    prev = (m2, g8, t)
            pm, pg, pt = prev
            v = pm[:].rearrange("p (k pb) -> p pb k", k=20, pb=16)
            nc.vector.tensor_reduce(
                out=x1parts[:, 32 * pg + 16 * pt:32 * pg + 16 * (pt + 1)],
                in_=v, axis=AX.X, op=ALU.max)
            nc.vector.tensor_scalar_add(out=x1parts[:], in0=x1parts[:], scalar1=b2rep[:])
            d["x1parts"] = x1parts

        # ------------------------------------------------------------------
        # Phase B2(s): x1 relayout + sq row, x1rep, conv2 kNN + gather issue
        # ------------------------------------------------------------------
        def phase_B2(s):
            d = st[s]
            x1parts = d["x1parts"]
            # x1c [16, 2048] = x1 [c, (g,t,ng,pb)]; sq2row [1, 2048] = -0.5|x1|^2
            x1c = set_p.tile([16, 2048], F32R, tag="x1c")
            x1v = x1c[:].rearrange("c (g t ng pb) -> c g t ng pb",
                                   g=8, t=2, ng=8, pb=16)
            for ng in range(8):
                nc.sync.dma_start(x1v[:, :, :, ng:ng + 1, :],
                                  x1parts[16 * ng:16 * ng + 16, :].bitcast(F32R))
            x1cb = big1_p.tile([16, 2048], BF16, tag="x1cb")
            nc.scalar.copy(x1cb[:], x1c[:])
            sq2row = big1_p.tile([1, 2048], BF16, tag="sq2row")
            for q in range(4):
                x1sq = big1_p.tile([16, 512], BF16, tag="x1sq")
                nc.scalar.activation(x1sq[:], x1cb[:, 512 * q:512 * (q + 1)], AF.Square)
                mq = pl_bg.tile([128, 512], F32, tag="bgps")
                nc.tensor.matmul(mq[0:1, :], lhsT=nh16b[:].bitcast(BF16), rhs=x1sq[:],
                                 start=True, stop=True)
                nc.scalar.copy(sq2row[:, 512 * q:512 * (q + 1)], mq[0:1, :])
            d["x1c"] = x1c
            d["sq2row"] = sq2row

            # x1rep [(ng,c), (g,node)] = x1 replicated into all 8 row-blocks
            x1rep = x1r_p.tile([128, 2048], F32, tag="x1rep")
            for ng in range(8):
                nc.sync.dma_start(x1rep[16 * ng:16 * ng + 16, :], x1c[:].bitcast(F32))

            f48 = f48_p.tile([48, 2048], F32R, tag="f48")
            nc.sync.dma_start(f48[32:48, :], x1c[:])
            d["f48"] = f48

            ixp = ix_p.tile([128, 320], I16, tag="ixp2")
            for g8 in range(8):
                for t in range(2):
                    knn_scores(x1cb, 16, sq2row[:, 256 * g8:256 * (g8 + 1)], g8, t, ixp)
            nc.vector.tensor_tensor(out=ixp[:].bitcast(U16), in0=ixp[:].bitcast(U16),
                                    in1=goffs[:].bitcast(U16), op=ALU.add)
            xjall = gp_p.tile([128, 5120], F32, tag="xjall")
            nc.gpsimd.ap_gather(xjall[:], x1rep[:], ixp[:],
                                channels=128, num_elems=2048, d=1, num_idxs=5120)
            d["xjall"] = xjall

        # ------------------------------------------------------------------
        # Phase C(s): conv2 q-matmuls + aggregation + f48 + lin1 + pool
        # ------------------------------------------------------------------
        def phase_C(s):
            d = st[s]
            x1c, f48, xjall = d["x1c"], d["f48"], d["xjall"]
            x2pa = set_p.tile([128, 256], F32, tag="x2pa")
            x2pb = set_p.tile([128, 256], F32, tag="x2pb")
            for g8 in range(8):
                for t in range(2):
                    xv = xjall[:, 640 * g8 + 320 * t:640 * g8 + 320 * (t + 1)]
                    qa = pl_ml.tile([128, 320], F32, tag="mlps")
                    nc.tensor.matmul(qa[:], lhsT=qbd_a[:], rhs=xv, start=True, stop=True)
                    qv = qa[:].rearrange("p (k pb) -> p pb k", k=20, pb=16)
                    nc.vector.tensor_reduce(
                        out=x2pa[:, 32 * g8 + 16 * t:32 * g8 + 16 * (t + 1)],
                        in_=qv, axis=AX.X, op=ALU.max)
                    qb = pl_ml.tile([128, 320], F32, tag="mlps")
                    nc.tensor.matmul(qb[:], lhsT=qbd_b[:], rhs=xv, start=True, stop=True)
                    qv = qb[:].rearrange("p (k pb) -> p pb k", k=20, pb=16)
                    nc.vector.tensor_reduce(
                        out=x2pb[:, 32 * g8 + 16 * t:32 * g8 + 16 * (t + 1)],
                        in_=qv, axis=AX.X, op=ALU.max)
            # linear term of conv2 (+bias) -> F48 rows 0..31 directly
            for g8 in range(8):
                l2 = pl_bg.tile([128, 512], F32, tag="bgps")
                nc.tensor.matmul(l2[0:32, 0:256], lhsT=wd16[:],
                                 rhs=x1c[:, 256 * g8:256 * (g8 + 1)].bitcast(F32),
                                 start=True, stop=True)
                nc.scalar.add(f48[0:32, 256 * g8:256 * (g8 + 1)], l2[0:32, 0:256], b2c2[:])
            x2t = big1_p.tile([32, 2048], F32, tag="x2t")
            x2tv = x2t[:].rearrange("c (g t ng pb) -> c g t ng pb",
                                    g=8, t=2, ng=8, pb=16)
            for ng in range(8):
                nc.sync.dma_start(x2tv[0:16, :, :, ng:ng + 1, :],
                                  x2pa[16 * ng:16 * ng + 16, :])
                nc.sync.dma_start(x2tv[16:32, :, :, ng:ng + 1, :],
                                  x2pb[16 * ng:16 * ng + 16, :])
            nc.vector.tensor_tensor(out=f48[0:32, :], in0=f48[0:32, :],
                                    in1=x2t[:], op=ALU.add)
            for g8 in range(8):
                pl = pl_sc.tile([128, 256], F32, tag="scps")
                nc.tensor.matmul(pl[:], lhsT=w1l[:], rhs=f48[:, 256 * g8:256 * (g8 + 1)],
                                 start=True, stop=True)
                nc.vector.tensor_reduce(out=pooledT[:, 8 * s + g8:8 * s + g8 + 1],
                                        in_=pl[:], axis=AX.X, op=ALU.max)
            st[s] = {}

        # ---------------- 4-stage pipelined emission ----------------
        for i in range(sets + 2):
            if i < sets:
                phase_A(i)
            if 1 <= i <= sets:
                phase_B1(i - 1)
                phase_B2(i - 1)
            if i >= 2:
                phase_C(i - 2)

        # ---------------- head MLP ----------------
        nc.vector.tensor_scalar_add(out=pooledT[:], in0=pooledT[:], scalar1=b1l[:])
        hd1 = pl_bg.tile([128, 512], F32, tag="bgps")
        nc.tensor.matmul(hd1[0:64, 0:G], lhsT=mw0[:], rhs=pooledT[:], start=True, stop=True)
        h1s = wp.tile([64, G], F32)
        nc.scalar.activation(h1s[:], hd1[0:64, 0:G], AF.Relu, bias=mb0[:])
        hd2 = pl_bg.tile([128, 512], F32, tag="bgps")
        nc.tensor.matmul(hd2[0:64, 0:G], lhsT=mw1[:], rhs=h1s[:], start=True, stop=True)
        h2s = wp.tile([64, G], F32)
        nc.scalar.activation(h2s[:], hd2[0:64, 0:G], AF.Relu, bias=mb1[:])
        hd3 = pl_bg.tile([128, 512], F32, tag="bgps")
        nc.tensor.matmul(hd3[0:1, 0:G], lhsT=mw2[:], rhs=h2s[:], start=True, stop=True)
        outs = wp.tile([1, G], F32)
        nc.vector.tensor_scalar_add(out=outs[:], in0=hd3[0:1, 0:G], scalar1=mb2[:])
        nc.sync.dma_start(out_d.ap(), outs[:])

    nc.compile()
    return nc


# ---------------------------------------------------------------------------
# Host-side input prep
# ---------------------------------------------------------------------------

def _blkdiag8(w):
    w = np.asarray(w, np.float32)
    n, m = w.shape
    out = np.zeros((8 * n, 8 * m), np.float32)
    for i in range(8):
        out[n * i:n * i + n, m * i:m * i + m] = w
    return out


def make_in_maps(inputs):
    x = np.asarray(inputs["x"], np.float32)
    c1_w0 = np.asarray(inputs["c1_w0"], np.float32)
    consts = {}
    consts["cw2"] = (c1_w0[:2] - c1_w0[2:4]).astype(np.float32)
    consts["b0c"] = np.asarray(inputs["c1_b0"], np.float32)[:, None]
    consts["bwrep"] = np.tile(c1_w0[2:4], (1, 8)).astype(np.float32)
    consts["w1bd"] = _blkdiag8(inputs["c1_w1"])
    consts["b1rep"] = np.tile(np.asarray(inputs["c1_b1"], np.float32), 8)[:, None]
    consts["w2bd"] = _blkdiag8(inputs["c1_w2"])
    consts["b2rep"] = np.tile(np.asarray(inputs["c1_b2"], np.float32), 8)[:, None]
    bdgf = np.zeros((128, 64), np.float32)
    for g in range(64):
        bdgf[2 * g, g] = -0.5
        bdgf[2 * g + 1, g] = -0.5
    consts["bdgf"] = bdgf
    consts["nh16b"] = np.full((16, 1), 0xBF00, np.uint16)   # bf16 -0.5
    consts["ones1b"] = np.full((1, 128), 0x3F80, np.uint16)  # bf16 1.0
    consts["goffs"] = np.tile(np.repeat(np.arange(8, dtype=np.int16) * 256, 40),
                              (128, 1))
    c2_w0 = np.asarray(inputs["c2_w0"], np.float32)
    consts["qbdA"] = _blkdiag8(c2_w0[16:32, 0:16])
    consts["qbdB"] = _blkdiag8(c2_w0[16:32, 16:32])
    consts["wd16"] = (c2_w0[0:16] - c2_w0[16:32]).astype(np.float32)
    consts["b2c2"] = np.asarray(inputs["c2_b0"], np.float32)[:, None]
    w1l = np.asarray(inputs["lin1_w"], np.float32)
    consts["w1l"] = np.concatenate([w1l[16:48], w1l[0:16]], axis=0)
    consts["b1l"] = np.asarray(inputs["lin1_b"], np.float32)[:, None]
    consts["mw0"] = np.asarray(inputs["m_w0"], np.float32)
    consts["mb0"] = np.asarray(inputs["m_b0"], np.float32)[:, None]
    consts["mw1"] = np.asarray(inputs["m_w1"], np.float32)
    consts["mb1"] = np.asarray(inputs["m_b1"], np.float32)[:, None]
    consts["mw2"] = np.asarray(inputs["m_w2"], np.float32)
    consts["mb2"] = np.asarray(inputs["m_b2"], np.float32)[:, None]

    in_maps = []
    npc = N * GPC
    for c in range(NCORES):
        xc = x[c * npc:(c + 1) * npc]
        xg = xc.reshape(GPC, N, 2)
        m = dict(consts)
        m["xtf"] = xg.transpose(0, 2, 1).reshape(128, 256).copy()
        m["xrows"] = xc.T.reshape(2, -1).copy()
        in_maps.append(m)
    return in_maps


_CACHED = {}


def _get_program(sets=8):
    if sets not in _CACHED:
        _CACHED[sets] = build_program(sets)
    return _CACHED[sets]


def run(inputs, trace=False, **kw):
    nc = _get_program(8)
    in_maps = make_in_maps(inputs)
    res = run_bass_kernel_spmd(nc, in_maps, list(range(NCORES)), trace=trace, **kw)
    out = np.concatenate([res.results[c]["out"].reshape(GPC) for c in range(NCORES)])
    return out.reshape(B, 1).astype(np.float32), res


def kernel(**inputs) -> np.ndarray:
    out, _ = run(inputs, trace=False)
    return out
